# revision 17
# baseline (speedup 1.0000x reference)
"""Trainium2 Bass kernel for nn_Conv2dGeneral (capsule-style 4x4-pose conv).

Math (from the reference):
  out[b,o,X,Y,u,w] = sum_{cin,kx,ky,v} Wm[(cin,kx,ky),o,u,v] * x[b,cin,2X+kx,2Y+ky,4v+w] + bias[o]

Mapped to the PE array as a single 1152-deep contraction:
  K = (cin, v)  x  9 accumulation chunks over (kx, ky)   [9 x 128]
  M = (o, u)                                              [128 PSUM partitions]
  N = (X, Y, w)                                           [676 per batch image]

Data-parallel across 8 NeuronCores on the batch dim (8 images per core).

Pipelining: weights + all 8 images are packed into ONE fp16 DRAM buffer per
core, streamed via column-range DMA chunks (img0 in row-quarters so compute
starts as early as possible, imgs 1-7 whole). Row/col 27 of x are dead
(stride-2 3-tap windows over 28 touch only 0..26) and dropped host-side.
The PE warms its HAM clock gate on a memset scratch region while chunk 0 is
in flight. Outputs are evicted per-group in fp16 (bias re-added host-side)
and shipped per-image by the SP engine AFTER all input triggers: HWDGE ring
FIFO then guarantees output traffic never delays input streaming.
"""

import numpy as np

B, CIN, COUT = 64, 32, 32
KK, STRIDE = 3, 2
WIN, HH = 28, 16
H = 4
WU = 27                          # used rows/cols (row 27 never read)
WOUT = (WIN - KK) // STRIDE + 1  # 13
NCORES = 8
BPC = B // NCORES                # batches per core
RCW = WU * WU * H                # 2916 free elements per (cin,v) partition
RL = WU * H                      # 108 elems per row
NOUT = WOUT * WOUT * H           # 676 outputs per (o,u) partition per image
XSPLIT0 = ((0, 2), (2, 2), (4, 4), (8, 5))  # img0: fine-grained X groups
XSPLIT = ((0, 4), (4, 4), (8, 5))           # imgs 1-7: X groups
WARMUP = 30                      # PE warm-up matmuls while chunk 0 streams in

OFF_X = 9 * 128                  # [wt(1152) | img0..7(2916 each)]
NELEM = OFF_X + BPC * RCW

# Per-group (image, t) -> (X0, nX, required chunk index).
# DMA chunks (elem ranges). img0 rows split [0,5) [5,9) [9,17) [17,27)
# matching XSPLIT0 needs (X rows 0-1 need x-rows 0-4, X 2-3 need 5-8, ...).
# Each chunk has its OWN completion semaphore: a single cumulative sem is
# racy because the 16 per-SDMA-engine increments of back-to-back DMAs
# interleave, so sem>=16 does not imply the FIRST dma finished. Per-engine
# FIFO order does make "chunk c done" imply all earlier chunks done.
_CHUNKS = [
    (0, OFF_X + 5 * RL),
    (OFF_X + 5 * RL, OFF_X + 9 * RL),
    (OFF_X + 9 * RL, OFF_X + 17 * RL),
    (OFF_X + 17 * RL, OFF_X + RCW),
]
for _b in range(1, BPC):
    _o = OFF_X + _b * RCW
    _CHUNKS.append((_o, _o + RCW))
NCHUNK = len(_CHUNKS)

GROUPS = []  # (b, X0, nX, chunk_needed)
for _t, (_x0, _nx) in enumerate(XSPLIT0):
    GROUPS.append((0, _x0, _nx, _t))
for _b in range(1, BPC):
    for _x0, _nx in XSPLIT:
        GROUPS.append((_b, _x0, _nx, _b + 3))
NG = len(GROUPS)                 # 25
# act_sem value after image b's last group is evicted
_IMG_DONE = {b: max(j for j, g in enumerate(GROUPS) if g[0] == b) + 1
             for b in range(BPC)}

_cache = {}


def _build_bass():
    """Raw-bass build (no Tile): this toolchain's walrus codegen allows only
    ONE sync-wait per instruction, so all cross-engine sync is explicit
    single-sem waits; ordering beyond that rides on hardware transitivity.

    Engines: SP triggers 11 input DMA chunks then 8 per-image output DMAs
    (same HWDGE ring: FIFO keeps outputs behind all inputs), PE runs 25
    accumulation groups of 9 matmuls (one per kernel tap), ACT evicts
    PSUM->SBUF in fp16.
    """
    from contextlib import ExitStack

    import concourse.bass as bass
    import concourse.mybir as mybir

    f32 = mybir.dt.float32
    f16 = mybir.dt.float16

    nc = bass.Bass()
    xin = nc.declare_dram_parameter("xin", [128, NELEM], f16, isOutput=False)
    o_d = nc.declare_dram_parameter("out", [BPC, 128, NOUT], f16, isOutput=True)

    with (
        ExitStack() as stack,
        nc.sbuf_tensor([128, NELEM], f16) as allt,
        nc.sbuf_tensor([128, BPC, NOUT], f16) as ot,
        nc.psum_tensor([128, 8, 512], f32) as ps,
        nc.semaphore("pe_sem") as pe_sem,
        nc.semaphore("act_sem") as act_sem,
        nc.semaphore("out_sem") as out_sem,
        nc.semaphore("warm_sem") as warm_sem,
        nc.Block(no_gpsimd_drain=True) as block,
    ):
        c_sems = [
            stack.enter_context(nc.semaphore(f"c_sem{i}")) for i in range(NCHUNK)
        ]
        wtr = allt[:, 0 : 9 * 128].rearrange("p (k m) -> p k m", k=9)

        @block.sync
        def _(sync):
            for c, (a0, a1) in enumerate(_CHUNKS):
                sync.dma_start(allt[:, a0:a1], xin[:, a0:a1]).then_inc(c_sems[c], 16)
            for b in range(BPC):
                # output ships as soon as its eviction is done; ring FIFO
                # keeps the transfer itself behind all input chunks
                sync.wait_ge(act_sem, _IMG_DONE[b])
                sync.dma_start(o_d[b], ot[:, b, :]).then_inc(out_sem, 16)
            sync.wait_ge(out_sem, 16 * BPC)

        @block.vector
        def _(vector):
            # Zero the warm-up operand region: reading never-written SBUF
            # trips the sim (and is unhealthy on hardware).
            vector.memset(ot[:, 0, :128], 0).then_inc(warm_sem, 1)

        @block.tensor
        def _(tensor):
            # Warm the PE HAM clock gate (cold = 1.2 GHz) on zeros while
            # chunk 0 (weights + img0 rows 0-4) streams in.
            tensor.wait_ge(warm_sem, 1)
            for i in range(WARMUP):
                tensor.matmul(
                    ps[:, 7, :128], ot[:, 0, :128], ot[:, 0, :128],
                    start=True, stop=True,
                )
            prev_need = -1
            for j, (b, X0, nX, need) in enumerate(GROUPS):
                if need > prev_need:
                    tensor.wait_ge(c_sems[need], 16)
                    prev_need = need
                if j >= 8:
                    # PSUM bank j%8 is free once ACT drained group j-8
                    tensor.wait_ge(act_sem, j - 7)
                gr = allt[:, OFF_X + b * RCW : OFF_X + (b + 1) * RCW].rearrange(
                    "p (r c w) -> p r c w", r=WU, c=WU
                )
                for kk in range(9):
                    kx, ky = divmod(kk, 3)
                    rhs = gr[
                        :,
                        2 * X0 + kx : 2 * X0 + kx + 2 * nX - 1 : 2,
                        ky : ky + 2 * WOUT - 1 : 2,
                        :,
                    ]
                    mm = tensor.matmul(
                        ps[:, j % 8, : nX * WOUT * H],
                        wtr[:, kk, :],
                        rhs,
                        start=(kk == 0),
                        stop=(kk == 8),
                    )
                mm.then_inc(pe_sem, 1)

        @block.scalar
        def _(scalar):
            for j, (b, X0, nX, _) in enumerate(GROUPS):
                off = X0 * WOUT * H
                scalar.wait_ge(pe_sem, j + 1)
                scalar.activation(
                    ot[:, b, off : off + nX * WOUT * H],
                    ps[:, j % 8, : nX * WOUT * H],
                    mybir.ActivationFunctionType.Copy,
                ).then_inc(act_sem, 1)

    return nc


def _prep_inputs(x, W, bias):
    """Build per-core [128, NELEM] fp16 input buffers.

    fp16: PE runs fp32 matmuls as LOW_HIGH double passes; fp16 is single-pass
    with fast-weight-load, and halves the dominant HBM traffic. Max rel err
    ~5e-4 at this contraction depth (fp32 PSUM accumulation).
    """
    x = np.asarray(x, dtype=np.float32)
    # xp[b, cin*4+v, (r*27+c)*4+w] = x[b,cin,r,c,4v+w], r/c < 27
    xp = np.ascontiguousarray(
        x.reshape(B, CIN, WIN, WIN, H, H)[:, :, :WU, :WU]
        .transpose(0, 1, 4, 2, 3, 5)
    ).reshape(B, CIN * H, RCW).astype(np.float16)
    # W: (1, 288, 32, 1, 1, 4, 4); p = cin*9 + kx*3 + ky
    # wt_sb[cin*4+v, kk*128 + o*4+u] = Wm[cin*9+kk, o, u, v]
    Wm = np.asarray(W, dtype=np.float32).reshape(CIN, KK * KK, COUT, H, H)
    wt_sb = np.ascontiguousarray(
        Wm.transpose(0, 4, 1, 2, 3)  # cin, v, kk, o, u
    ).reshape(128, 9 * 128).astype(np.float16)
    bufs = []
    for core in range(NCORES):
        shard = xp[core * BPC : (core + 1) * BPC]  # (BPC, 128, RCW)
        bufs.append(
            np.ascontiguousarray(
                np.concatenate(
                    [wt_sb, shard.transpose(1, 0, 2).reshape(128, BPC * RCW)],
                    axis=1,
                )
            )
        )
    return bufs


def _make_in_maps(x, W, bias):
    return [{"xin": buf} for buf in _prep_inputs(x, W, bias)]


def _unprep_output(full, bias):
    # full: (B, 128, NOUT) fp16 with partition o*4+u, free (X, Y, w).
    # Bias (a per-channel constant) is added host-side to keep the device
    # eviction a plain fp16 Copy.
    out = (
        full.astype(np.float32)
        .reshape(B, COUT, H, WOUT, WOUT, H)
        .transpose(0, 1, 3, 4, 2, 5)
        .reshape(B, COUT, WOUT, WOUT, HH)
    )
    out += np.asarray(bias, dtype=np.float32).reshape(1, COUT, 1, 1, 1)
    return np.ascontiguousarray(out)


def run_device(in_maps, trace=False, tmpdir=None):
    from concourse.bass_utils import run_bass_kernel_spmd

    if "nc" not in _cache:
        _cache["nc"] = _build_bass()
    return run_bass_kernel_spmd(
        _cache["nc"], in_maps, list(range(NCORES)), trace=trace, tmpdir=tmpdir
    )


def kernel(x, W, bias):
    in_maps = _make_in_maps(x, W, bias)
    res = run_device(in_maps, trace=False)
    full = np.concatenate(
        [np.asarray(res.results[i]["out"]) for i in range(NCORES)], axis=0
    )
    return _unprep_output(full, bias)
